# revision 30
# baseline (speedup 1.0000x reference)
"""Trainium2 Bass kernel for CompressedCausalAttention.

Reference computation (S=1024, B=4, C=1024, H=16, CC=64):
    qkv = (x + pe) @ Wqkv.T + bqkv
    q, k, v = split(qkv); reshape to [s, b, H, CC]
    qk = einsum('sbhc,tbhc->stbh', q, k) / sqrt(CC)
    mask = content_mask[:,:,:,None] | padding_mask[None,:,:,None]
    p = softmax(where(mask, -inf, qk), axis=1)
    out = einsum('stbh,tbhc->sbhc', p, v).reshape(s,b,c) @ Wo.T + bo

Sharding: 8 cores = (batch b in 0..3) x (head-group hg in 0..1, 8 heads each).
Each core projects q/k/v for its 8 heads of its batch (x+pe fused on host),
computes attention with scores in transposed [t, s] layout (softmax row-sum
comes free from an appended ones-column in V; no max-subtraction needed at
these magnitudes), then computes a PARTIAL output projection over its local
512 channels producing [S, C] bf16.  No collective: the host sums the two
partials of each batch and adds the combined bias bo + Wo @ bv (the v-bias
contribution is exact because softmax rows sum to 1 after normalization).
q/k biases are applied by per-partition vector adds during the PSUM->SBUF
move, so there are no bias matmuls on the PE at all.

Pipeline (causal): the exp of the attention scores is the scalar-engine
bottleneck (~40 activations of ~1.3us), so score computation + exp for
early query pairs is overlapped with later projection waves: after q/k of
token-half 0 are projected, pairs 0-1 scores/exp run interleaved with the
half-1 q/k projection matmuls; pair 2 overlaps the V wave; only pair 3's
exp remains in the PV/output phase.  PSUM: projection waves use 4 banks
(except the first fused q+k wave: 8), score chunks 4, PV/out phase 4.
"""

import os
import sys
import types

import ml_dtypes
import numpy as np

_SO_PATH = "/opt/axon/libaxon_pjrt.so"


def _install_ntff_shim():
    """Make `antenv.axon_hooks` importable so trace=True works under axon."""
    try:
        from antenv.axon_hooks import set_axon_ntff_profile_hook  # noqa: F401

        return
    except ImportError:
        pass
    try:
        import antenv
        import trn_agent_boot.trn_boot as tb
    except ImportError:
        return
    mod = types.ModuleType("antenv.axon_hooks")
    _hook = [None]
    mod.set_axon_ntff_profile_hook = lambda h: _hook.__setitem__(0, h)
    mod.get_axon_ntff_profile_hook = lambda: _hook[0]
    sys.modules["antenv.axon_hooks"] = mod
    antenv.axon_hooks = mod
    if os.path.exists(_SO_PATH):
        mod.set_axon_ntff_profile_hook(tb._ntff_profile_via_ctypes(_SO_PATH))


_install_ntff_shim()

import concourse.bass as bass  # noqa: E402
import concourse.tile as tile  # noqa: E402
from concourse import bacc, mybir  # noqa: E402
from concourse.bass_utils import run_bass_kernel_spmd  # noqa: E402

S = 1024
B = 4
C = 1024
H = 16
CC = 64
HG = 8  # heads per core
F = HG * CC  # 512 features per core for each of q/k/v
P = 128
NQ = S // P  # 8 query tiles
NT = S // P  # 8 key tiles
KT = C // P  # 8 contraction tiles
TEMP = 1.0 / 8.0

DTB = mybir.dt.bfloat16
DTF = mybir.dt.float32
BF16 = ml_dtypes.bfloat16

N_CORES = 8

_NC_CACHE = {}
WARM_PRE = 12  # warmup matmuls before the first projection wave
LAST_RESULT = None  # BassKernelResults of the most recent run (for profiling)


def _build(causal: bool):
    """Build the SPMD program. `causal`: triangular block skipping + tril
    mask on diagonal score blocks; padding is folded into V (padded rows and
    the denominator column are zeroed). Otherwise a full [t, s] 0/1 mask
    input is applied per score block."""
    nc = bacc.Bacc("TRN2", target_bir_lowering=False, debug=False,
                   num_devices=N_CORES)

    xpeT = nc.dram_tensor("xpeT", [C, S], DTB, kind="ExternalInput")
    wqkvT = nc.dram_tensor("wqkvT", [C, 3 * F], DTB, kind="ExternalInput")
    woT = nc.dram_tensor("woT", [F, C], DTB, kind="ExternalInput")
    bqk_col = nc.dram_tensor("bqk_col", [P, 8], DTF, kind="ExternalInput")
    if causal:
        pad01 = nc.dram_tensor("pad01", [P, NT], DTF, kind="ExternalInput")
        pad_colb = nc.dram_tensor("pad_colb", [P, NT, HG], DTB,
                                  kind="ExternalInput")
        maskT = None
    else:
        pad01 = None
        maskT = nc.dram_tensor("maskT", [P, NT, S], DTB, kind="ExternalInput")
    out_h = nc.dram_tensor("out", [S, C], DTB, kind="ExternalOutput")

    tril_np = np.triu(np.ones((P, P))).astype(BF16)  # keep t <= s
    tril_dram = nc.inline_tensor(tril_np, name="tril_const")
    ident_dram = nc.inline_tensor(np.eye(P).astype(BF16), name="ident_const")

    from contextlib import ExitStack

    with tile.TileContext(nc) as tc, ExitStack() as ctx:
        cpool = ctx.enter_context(tc.tile_pool(name="const", bufs=1))
        pp = ctx.enter_context(tc.tile_pool(name="persist", bufs=1))

        xpe = pp.tile([P, KT, S], DTB)
        wq_b = pp.tile([P, KT, 3 * F], DTB)
        wo_b = pp.tile([P, 4, C], DTB)
        q_t = pp.tile([P, F // P, S], DTB)
        k_t = pp.tile([P, F // P, S], DTB)
        v_t = pp.tile([P, NT, HG * (CC + 1)], DTB)
        attn_l = pp.tile([P, F // P, S], DTB)     # attn^T local [hc, tok]

        # ---- input DMAs, ordered so the first QK wave's tiles land first:
        # per kt: xpe token-half0 (sync) + q/k weight cols (scalar); then
        # xpe token-half1; v weight cols + wo + consts on gpsimd ----
        for kt in range(KT):
            nc.sync.dma_start(xpe[:, kt, 0:S // 2],
                              xpeT[P * kt:P * (kt + 1), 0:S // 2])
            # alternate q/k weight tiles across two queues so consecutive
            # kt tiles land in parallel and the first wave never starves
            eng = nc.scalar if kt % 2 == 0 else nc.gpsimd
            eng.dma_start(wq_b[:, kt, 0:2 * F],
                          wqkvT[P * kt:P * (kt + 1), 0:2 * F])
        for kt in range(KT):
            nc.sync.dma_start(xpe[:, kt, S // 2:S],
                              xpeT[P * kt:P * (kt + 1), S // 2:S])

        tril = cpool.tile([P, P], DTB)
        nc.gpsimd.dma_start(tril[:], tril_dram[:])
        ident = cpool.tile([P, P], DTB)
        nc.gpsimd.dma_start(ident[:], ident_dram[:])
        bqk_t = cpool.tile([P, 8], DTF)
        nc.gpsimd.dma_start(bqk_t[:], bqk_col[:])
        if causal:
            pad_t = cpool.tile([P, NT], DTF)
            nc.gpsimd.dma_start(pad_t[:], pad01[:])
            pad_cb = cpool.tile([P, NT, HG], DTB)
            nc.gpsimd.dma_start(pad_cb[:], pad_colb[:])
        else:
            m_t = pp.tile([P, NT, S], DTB)
            for tj in range(NT):
                nc.gpsimd.dma_start(m_t[:, tj, :], maskT[:, tj, :])
        # delay the v-weight and wo DMA issue until the critical projection
        # inputs have landed: a tiny gpsimd copy depending on the last
        # xpe half-0 (resp. half-1) tile holds back the queue so the DMA
        # engines don't steal bandwidth from the first QK wave's stream
        dly = cpool.tile([1, 2], DTB)
        nc.gpsimd.tensor_copy(dly[0:1, 0:1], xpe[0:1, KT - 1, 511:512])
        for kt in range(KT):
            nc.gpsimd.dma_start(wq_b[:, kt, 2 * F:3 * F],
                                wqkvT[P * kt:P * (kt + 1), 2 * F:3 * F])
        nc.gpsimd.tensor_copy(dly[0:1, 1:2], xpe[0:1, KT - 1, 1023:1024])
        for kc in range(4):
            nc.gpsimd.dma_start(wo_b[:, kc, :], woT[P * kc:P * (kc + 1), :])

        warm_sta = cpool.tile([P, P], DTB)
        nc.vector.memset(warm_sta[:], 0.0)
        warm_rhs = cpool.tile([P, 256], DTB)
        nc.vector.memset(warm_rhs[:], 0.0)

        # doubled tril (for masking both heads of a pair in one op)
        tril2 = cpool.tile([P, 2, P], DTB)
        for x in range(2):
            nc.vector.tensor_copy(tril2[:, x, :], tril[:])
        if not causal:
            m2_t = pp.tile([P, 2, NT, S], DTB)
            for x in range(2):
                nc.vector.tensor_copy(m2_t[:, x, :, :], m_t[:])

        # ---- PE warmup: dummy matmuls during the input-DMA window release
        # the HAM clock gate and keep the p-state ramp warm before the
        # projections start ----
        with tc.tile_pool(name="warmps", bufs=1, space="PSUM") as warmps:
            wps = warmps.tile([P, 256], DTF)
            for _ in range(WARM_PRE):
                nc.tensor.matmul(wps[:], warm_sta[:], warm_rhs[:])

        v3 = v_t[:].rearrange("p n (h c) -> p n h c", c=CC + 1)
        if causal:
            nc.vector.tensor_copy(v3[:, :, :, CC], pad_cb[:])
        else:
            nc.vector.memset(v3[:, :, :, CC], 1.0)

        ep = ctx.enter_context(tc.tile_pool(name="ep", bufs=3))
        ptp = ctx.enter_context(tc.tile_pool(name="ptp",
                                             bufs=1 if causal else 4))
        osbp = ctx.enter_context(tc.tile_pool(name="osb", bufs=4))

        phase_pj = ExitStack()
        projps = phase_pj.enter_context(tc.tile_pool(name="projps", bufs=1,
                                                     space="PSUM"))

        def qk_half_wave(which, nh, pool, tagpfx):
            # project q (which=0) or k (which=1), 4 feature tiles, token
            # half nh; kt-outer so matmuls stream behind the weight DMAs
            tiles = [pool.tile([P, S // 2], DTF, tag=f"{tagpfx}{i}",
                               name=f"ps{tagpfx}{i}") for i in range(4)]
            for kt in range(KT):
                step = []
                for ft in range(4):
                    step.append(nc.tensor.matmul(
                        tiles[ft][:],
                        wq_b[:, kt, F * which + P * ft:F * which + P * (ft + 1)],
                        xpe[:, kt, (S // 2) * nh:(S // 2) * (nh + 1)],
                        start=(kt == 0), stop=(kt == KT - 1),
                    ))
                yield step
            dst = q_t if which == 0 else k_t
            for ft in range(4):
                nc.vector.tensor_scalar_add(
                    dst[:, ft, (S // 2) * nh:(S // 2) * (nh + 1)],
                    tiles[ft][:], bqk_t[:, 4 * which + ft:4 * which + ft + 1])
                yield

        def v_wave():
            # tt-outer: each token tile finishes its contraction and hands
            # off to the vector epilogue while the PE moves to the next tile
            for tt in range(8):
                vt = projps.tile([P, F], DTF, tag=f"pj{tt % 4}",
                                 name=f"psv{tt}")
                for kt in range(KT):
                    nc.tensor.matmul(
                        vt[:], xpe[:, kt, P * tt:P * (tt + 1)],
                        wq_b[:, kt, 2 * F:3 * F],
                        start=(kt == 0), stop=(kt == KT - 1),
                    )
                if causal:
                    nc.vector.tensor_scalar_mul(
                        v3[:, tt, :, 0:CC],
                        vt[:].rearrange("p (h c) -> p h c", c=CC),
                        pad_t[:, tt:tt + 1],
                    )
                else:
                    nc.vector.tensor_copy(
                        v3[:, tt, :, 0:CC],
                        vt[:].rearrange("p (h c) -> p h c", c=CC),
                    )
                yield

        pts = {}  # (qp, hp) -> exp'd score tile, [P, 2, n_t, 2P]

        def score_exp(qp, ctj, scpool):
            # scores + exp for query pair qp; the exp'd probabilities land
            # in SBUF tiles that survive until pv_out(qp) consumes them
            q0, q1 = 2 * qp, 2 * qp + 1
            n_t = q1 + 1 if causal else NT
            for hp in range(HG // 2):
                ft = hp
                pt = ptp.tile([P, 2, n_t, 2 * P], DTB,
                              tag=f"pt{qp}_{hp}" if causal else "pt",
                              name=f"pt{qp}_{hp}")
                pts[(qp, hp)] = pt
                for c0 in range(0, n_t, ctj):
                    cn = min(ctj, n_t - c0)
                    scp = scpool.tile([P, 2, ctj, 2 * P], DTF, tag="scp",
                                      name="scp")
                    for tj in range(c0, c0 + cn):
                        nc.tensor.matmul(
                            scp[:, 0, tj - c0, :],
                            k_t[0:CC, ft, P * tj:P * (tj + 1)],
                            q_t[0:CC, ft, 2 * P * qp:2 * P * (qp + 1)],
                        )
                        nc.tensor.matmul(
                            scp[:, 1, tj - c0, :],
                            k_t[CC:P, ft, P * tj:P * (tj + 1)],
                            q_t[CC:P, ft, 2 * P * qp:2 * P * (qp + 1)],
                        )
                    nc.scalar.activation(
                        pt[:, :, c0:c0 + cn, :], scp[:, :, 0:cn, :],
                        mybir.ActivationFunctionType.Exp, scale=TEMP)
                    yield

        def ramp(n):
            # dummy matmuls that keep the PE p-state ramp warm across
            # unavoidable dependency waits (a reset costs 2x clock for 3us)
            rt = rampps.tile([P, 256], DTF, tag="ramp", name="ramp")
            for _ in range(n):
                nc.tensor.matmul(rt[:], warm_sta[:], warm_rhs[:])

        def pv_out(qp, keep_warm=False):
            # mask, PV, normalization, transpose into attn_l for pair qp
            q0, q1 = 2 * qp, 2 * qp + 1
            for iq, qi in enumerate((q0, q1)):
                nt_i = qi + 1 if causal else NT
                # head stride padded to 128 f32 so no PV output window
                # crosses a PSUM bank boundary (2 banks, 4 heads per bank)
                out_ab = ops_pool.tile([P, HG, P], DTF,
                                       tag="outab", name="out_ab")
                for hp in range(HG // 2):
                    pt = pts[(qp, hp)]
                    if causal:
                        nc.vector.tensor_mul(
                            pt[:, :, qi, P * iq:P * (iq + 1)],
                            pt[:, :, qi, P * iq:P * (iq + 1)], tril2[:])
                    else:
                        for tj in range(nt_i):
                            nc.vector.tensor_mul(
                                pt[:, :, tj, P * iq:P * (iq + 1)],
                                pt[:, :, tj, P * iq:P * (iq + 1)],
                                m2_t[:, :, tj, P * qi:P * (qi + 1)])
                    anchor = None
                    for x, h in ((0, 2 * hp), (1, 2 * hp + 1)):
                        for tj in range(nt_i):
                            anchor = nc.tensor.matmul(
                                out_ab[:, h, 0:CC + 1],
                                pt[:, x, tj, P * iq:P * (iq + 1)],
                                v_t[:, tj, (CC + 1) * h:(CC + 1) * (h + 1)],
                                start=(tj == 0), stop=(tj == nt_i - 1),
                            )
                    yield anchor
                if keep_warm:
                    ramp(3)
                # normalization epilogue for this query tile (single
                # reciprocal + single scaled copy over all 8 heads)
                rec = ep.tile([P, HG], DTF, tag="rec", name="rec")
                nc.vector.reciprocal(rec[:], out_ab[:, :, CC])
                attn_s = ep.tile([P, F], DTB, tag="attn_s", name="attn_s")
                recb = rec[:, :, None].broadcast_to([P, HG, CC])
                nc.vector.tensor_tensor(
                    attn_s[:].rearrange("p (h c) -> p h c", c=CC),
                    out_ab[:, :, 0:CC],
                    recb,
                    mybir.AluOpType.mult,
                )
                tp = tpps.tile([P, HG // 2, P], DTB, tag="tp", name="tp")
                anchor = None
                for hp in range(HG // 2):
                    anchor = nc.tensor.transpose(tp[:, hp, :],
                                                 attn_s[:, P * hp:P * (hp + 1)],
                                                 ident[:])
                for hp in range(HG // 2):
                    nc.vector.tensor_copy(attn_l[:, hp, P * qi:P * (qi + 1)],
                                          tp[:, hp, :])
                yield anchor

        def out_proj(mt):
            # partial projection: local 512 channels (4 contraction tiles)
            # -> all C output columns, emitted in two 512-col halves
            psf = fo.tile([P, C // 2], DTF, tag="fo", name="psf")
            for h in range(2):
                step = []
                for kc in range(4):
                    step.append(nc.tensor.matmul(
                        psf[:], attn_l[:, kc, P * mt:P * (mt + 1)],
                        wo_b[:, kc, (C // 2) * h:(C // 2) * (h + 1)],
                        start=(kc == 0), stop=(kc == 3),
                    ))
                yield step
                osb = osbp.tile([P, C // 2], DTB, tag="osb", name="osb")
                # ACT is idle once the exps drain (~late pv2), so the late
                # tiles' PSUM->bf16 casts go there (they would otherwise
                # queue behind the pv3 epilogue on vector); the final tile
                # is emitted in quarter-width chunks to shorten the tail
                nq = 2 if mt == 7 else 1
                qw = (C // 2) // nq
                for qx in range(nq):
                    if mt >= 4:
                        nc.scalar.copy(osb[:, qw * qx:qw * (qx + 1)],
                                       psf[:, qw * qx:qw * (qx + 1)])
                    else:
                        nc.vector.tensor_copy(osb[:, qw * qx:qw * (qx + 1)],
                                              psf[:, qw * qx:qw * (qx + 1)])
                    nc.sync.dma_start(
                        out_h[P * mt:P * (mt + 1),
                              (C // 2) * h + qw * qx:(C // 2) * h + qw * (qx + 1)],
                        osb[:, qw * qx:qw * (qx + 1)])

        def weave(main_gen, fillers):
            """Run main_gen; after each of its yields, advance the current
            filler generator by one step."""
            for _ in main_gen:
                while fillers:
                    try:
                        next(fillers[0])
                        break
                    except StopIteration:
                        fillers.pop(0)
            for fg in fillers:
                for _ in fg:
                    pass
            fillers.clear()

        def run(gen):
            for _ in gen:
                pass

        def chain(*gens):
            for g in gens:
                yield from g

        def interleave(ga, gb):
            ita, itb = iter(ga), iter(gb)
            while True:
                done = 0
                for it in (ita, itb):
                    try:
                        next(it)
                    except StopIteration:
                        done += 1
                if done == 2:
                    return
                yield

        if causal:
            # fused first wave: q and k for token half 0 (8 PSUM banks)
            with tc.tile_pool(name="pkps", bufs=1, space="PSUM") as pkps:
                run(interleave(qk_half_wave(0, 0, projps, "pj"),
                               qk_half_wave(1, 0, pkps, "pk")))
            # scores/exp of pairs 0-2 overlap the half-1 q/k and V waves
            with tc.tile_pool(name="scps", bufs=2, space="PSUM") as scps:
                weave(chain(score_exp(0, 2, scps), score_exp(1, 2, scps)),
                      [qk_half_wave(0, 1, projps, "pj"),
                       qk_half_wave(1, 1, projps, "pj")])
                weave(score_exp(2, 2, scps), [v_wave()])
            phase_pj.close()
            # PV / normalization / output projections; pair-3 scores/exp and
            # the out_projs fill PE bubbles
            ops_pool = ctx.enter_context(tc.tile_pool(name="ops", bufs=1,
                                                      space="PSUM"))
            tpps = ctx.enter_context(tc.tile_pool(name="tpps", bufs=1,
                                                  space="PSUM"))
            fo = ctx.enter_context(tc.tile_pool(name="fo", bufs=1,
                                                space="PSUM"))
            scps2 = ctx.enter_context(tc.tile_pool(name="scps2", bufs=1,
                                                   space="PSUM"))
            rampps = ctx.enter_context(tc.tile_pool(name="rampps", bufs=1,
                                                    space="PSUM"))
            se3 = score_exp(3, 2, scps2)
            next(se3)
            next(se3)
            weave(chain(pv_out(0), pv_out(1), pv_out(2)),
                  [se3,
                   out_proj(0), out_proj(1), out_proj(2), out_proj(3)])
            weave(pv_out(3, keep_warm=True),
                  [out_proj(4), out_proj(5), out_proj(6)])
            ramp(6)
            run(out_proj(7))
        else:
            # simple sequential structure (correctness fallback)
            with tc.tile_pool(name="pkps", bufs=1, space="PSUM") as pkps:
                run(interleave(qk_half_wave(0, 0, projps, "pj"),
                               qk_half_wave(1, 0, pkps, "pk")))
            run(qk_half_wave(0, 1, projps, "pj"))
            run(qk_half_wave(1, 1, projps, "pj"))
            run(v_wave())
            phase_pj.close()
            scps = ctx.enter_context(tc.tile_pool(name="scps", bufs=2,
                                                  space="PSUM"))
            ops_pool = ctx.enter_context(tc.tile_pool(name="ops", bufs=1,
                                                      space="PSUM"))
            tpps = ctx.enter_context(tc.tile_pool(name="tpps", bufs=1,
                                                  space="PSUM"))
            fo = ctx.enter_context(tc.tile_pool(name="fo", bufs=1,
                                                space="PSUM"))
            for qp in range(4):
                run(score_exp(qp, 2, scps))
                run(pv_out(qp))
            for mt in range(8):
                run(out_proj(mt))

    nc.compile()
    return nc


def _get_nc(causal: bool):
    if causal not in _NC_CACHE:
        _NC_CACHE[causal] = _build(causal)
    return _NC_CACHE[causal]


def kernel(x, pe, content_mask, padding_mask, Wqkv, bqkv, Wo, bo):
    global LAST_RESULT
    x = np.asarray(x, dtype=np.float32)
    pe = np.asarray(pe, dtype=np.float32)
    content_mask = np.asarray(content_mask, dtype=bool)
    padding_mask = np.asarray(padding_mask, dtype=bool)
    Wqkv = np.asarray(Wqkv, dtype=np.float32)
    bqkv = np.asarray(bqkv, dtype=np.float32)
    Wo = np.asarray(Wo, dtype=np.float32)
    bo = np.asarray(bo, dtype=np.float32)
    assert x.shape == (S, B, C) and Wqkv.shape == (3 * C, C)

    causal_2d = np.triu(np.ones((S, S), dtype=bool), 1)
    causal = np.array_equal(content_mask,
                            np.broadcast_to(causal_2d[:, :, None], (S, S, B)))

    nc = _get_nc(causal)

    xpe_full = x + pe
    bv_full = bqkv[2 * C:3 * C]
    obias = bo + Wo @ bv_full  # [C], f32

    in_maps = []
    for core in range(N_CORES):
        b, hg = core // 2, core % 2
        m = {
            "xpeT": np.ascontiguousarray(xpe_full[:, b, :].T.astype(BF16)),
        }
        rows = np.concatenate([
            np.arange(F * hg, F * (hg + 1)),
            np.arange(C + F * hg, C + F * (hg + 1)),
            np.arange(2 * C + F * hg, 2 * C + F * (hg + 1)),
        ])
        m["wqkvT"] = np.ascontiguousarray(Wqkv[rows, :].T.astype(BF16))
        m["woT"] = np.ascontiguousarray(
            Wo[:, F * hg:F * (hg + 1)].T.astype(BF16))
        bq = bqkv[F * hg:F * (hg + 1)]
        bk = bqkv[C + F * hg:C + F * (hg + 1)]
        bqk = np.concatenate([bq, bk]).reshape(8, P).T  # [P, 8]
        m["bqk_col"] = np.ascontiguousarray(bqk.astype(np.float32))
        if causal:
            keep = (~padding_mask[:, b]).astype(np.float32)  # [S]
            m["pad01"] = np.ascontiguousarray(keep.reshape(NT, P).T)
            m["pad_colb"] = np.ascontiguousarray(np.broadcast_to(
                m["pad01"][:, :, None], (P, NT, HG)).astype(BF16))
        else:
            keep2d = ~(content_mask[:, :, b] | padding_mask[None, :, b])  # [s, t]
            mT = keep2d.T.astype(BF16)  # [t, s]
            m["maskT"] = np.ascontiguousarray(mT.reshape(NT, P, S).transpose(1, 0, 2))
        in_maps.append(m)

    trace = bool(os.environ.get("BASS_KERNEL_TRACE"))
    res = run_bass_kernel_spmd(nc, in_maps, core_ids=list(range(N_CORES)),
                               trace=trace)
    LAST_RESULT = res

    out = np.empty((S, B, C), dtype=np.float32)
    for b in range(B):
        p0 = res.results[2 * b]["out"].astype(np.float32)
        p1 = res.results[2 * b + 1]["out"].astype(np.float32)
        out[:, b, :] = p0 + p1 + obias[None, :]
    return out


# revision 33
# speedup vs baseline: 1.0919x; 1.0919x over previous
"""Trainium2 Bass kernel for CompressedCausalAttention.

Reference computation (S=1024, B=4, C=1024, H=16, CC=64):
    qkv = (x + pe) @ Wqkv.T + bqkv
    q, k, v = split(qkv); reshape to [s, b, H, CC]
    qk = einsum('sbhc,tbhc->stbh', q, k) / sqrt(CC)
    mask = content_mask[:,:,:,None] | padding_mask[None,:,:,None]
    p = softmax(where(mask, -inf, qk), axis=1)
    out = einsum('stbh,tbhc->sbhc', p, v).reshape(s,b,c) @ Wo.T + bo

Sharding: 8 cores = (batch b in 0..3) x (head-group hg in 0..1, 8 heads each).
Each core projects q/k/v for its 8 heads of its batch (x+pe fused on host),
computes attention with scores in transposed [t, s] layout (softmax row-sum
comes free from an appended ones-column in V; no max-subtraction needed at
these magnitudes), then computes a PARTIAL output projection over its local
512 channels producing [S, C] bf16.  No collective: the host sums the two
partials of each batch and adds the combined bias bo + Wo @ bv (the v-bias
contribution is exact because softmax rows sum to 1 after normalization).
q/k biases are applied by per-partition vector adds during the PSUM->SBUF
move, so there are no bias matmuls on the PE at all.

Pipeline (causal): the exp of the attention scores is the scalar-engine
bottleneck (~40 activations of ~1.3us), so score computation + exp for
early query pairs is overlapped with later projection waves: after q/k of
token-half 0 are projected, pairs 0-1 scores/exp run interleaved with the
half-1 q/k projection matmuls; pair 2 overlaps the V wave; only pair 3's
exp remains in the PV/output phase.  PSUM: projection waves use 4 banks
(except the first fused q+k wave: 8), score chunks 4, PV/out phase 4.
"""

import os
import sys
import types

import ml_dtypes
import numpy as np

_SO_PATH = "/opt/axon/libaxon_pjrt.so"


def _install_ntff_shim():
    """Make `antenv.axon_hooks` importable so trace=True works under axon."""
    try:
        from antenv.axon_hooks import set_axon_ntff_profile_hook  # noqa: F401

        return
    except ImportError:
        pass
    try:
        import antenv
        import trn_agent_boot.trn_boot as tb
    except ImportError:
        return
    mod = types.ModuleType("antenv.axon_hooks")
    _hook = [None]
    mod.set_axon_ntff_profile_hook = lambda h: _hook.__setitem__(0, h)
    mod.get_axon_ntff_profile_hook = lambda: _hook[0]
    sys.modules["antenv.axon_hooks"] = mod
    antenv.axon_hooks = mod
    if os.path.exists(_SO_PATH):
        mod.set_axon_ntff_profile_hook(tb._ntff_profile_via_ctypes(_SO_PATH))


_install_ntff_shim()

import concourse.bass as bass  # noqa: E402
import concourse.tile as tile  # noqa: E402
from concourse import bacc, mybir  # noqa: E402
from concourse.bass_utils import run_bass_kernel_spmd  # noqa: E402

S = 1024
B = 4
C = 1024
H = 16
CC = 64
HG = 8  # heads per core
F = HG * CC  # 512 features per core for each of q/k/v
P = 128
NQ = S // P  # 8 query tiles
NT = S // P  # 8 key tiles
KT = C // P  # 8 contraction tiles
TEMP = 1.0 / 8.0

DTB = mybir.dt.bfloat16
DTF = mybir.dt.float32
BF16 = ml_dtypes.bfloat16

N_CORES = 8

_NC_CACHE = {}
WARM_PRE = 12  # warmup matmuls before the first projection wave
LAST_RESULT = None  # BassKernelResults of the most recent run (for profiling)


def _build(causal: bool):
    """Build the SPMD program. `causal`: triangular block skipping + tril
    mask on diagonal score blocks; padding is folded into V (padded rows and
    the denominator column are zeroed). Otherwise a full [t, s] 0/1 mask
    input is applied per score block."""
    nc = bacc.Bacc("TRN2", target_bir_lowering=False, debug=False,
                   num_devices=N_CORES)

    xpeT = nc.dram_tensor("xpeT", [C, S], DTB, kind="ExternalInput")
    wqkvT = nc.dram_tensor("wqkvT", [C, 3 * F], DTB, kind="ExternalInput")
    woT = nc.dram_tensor("woT", [F, C], DTB, kind="ExternalInput")
    bqk_col = nc.dram_tensor("bqk_col", [P, 8], DTF, kind="ExternalInput")
    if causal:
        pad01 = nc.dram_tensor("pad01", [P, NT], DTF, kind="ExternalInput")
        pad_colb = nc.dram_tensor("pad_colb", [P, NT, HG], DTB,
                                  kind="ExternalInput")
        maskT = None
    else:
        pad01 = None
        maskT = nc.dram_tensor("maskT", [P, NT, S], DTB, kind="ExternalInput")
    out_h = nc.dram_tensor("out", [S, C], DTB, kind="ExternalOutput")

    tril_np = np.triu(np.ones((P, P))).astype(BF16)  # keep t <= s
    tril_dram = nc.inline_tensor(tril_np, name="tril_const")
    ident_dram = nc.inline_tensor(np.eye(P).astype(BF16), name="ident_const")

    from contextlib import ExitStack

    with tile.TileContext(nc) as tc, ExitStack() as ctx:
        cpool = ctx.enter_context(tc.tile_pool(name="const", bufs=1))
        pp = ctx.enter_context(tc.tile_pool(name="persist", bufs=1))

        xpe = pp.tile([P, KT, S], DTB)
        wq_b = pp.tile([P, KT, 3 * F], DTB)
        wo_b = pp.tile([P, 4, C], DTB)
        q_t = pp.tile([P, F // P, S], DTB)
        k_t = pp.tile([P, F // P, S], DTB)
        v_t = pp.tile([P, NT, HG * (CC + 1)], DTB)
        attn_l = pp.tile([P, F // P, S], DTB)     # attn^T local [hc, tok]

        # ---- input DMAs, ordered so the first QK wave's tiles land first:
        # per kt: xpe token-half0 (sync) + q/k weight cols (scalar); then
        # xpe token-half1; v weight cols + wo + consts on gpsimd ----
        for kt in range(KT):
            nc.sync.dma_start(xpe[:, kt, 0:S // 2],
                              xpeT[P * kt:P * (kt + 1), 0:S // 2])
            # alternate q/k weight tiles across two queues so consecutive
            # kt tiles land in parallel and the first wave never starves
            eng = nc.scalar if kt % 2 == 0 else nc.gpsimd
            eng.dma_start(wq_b[:, kt, 0:2 * F],
                          wqkvT[P * kt:P * (kt + 1), 0:2 * F])
        for kt in range(KT):
            nc.sync.dma_start(xpe[:, kt, S // 2:S],
                              xpeT[P * kt:P * (kt + 1), S // 2:S])

        tril = cpool.tile([P, P], DTB)
        nc.gpsimd.dma_start(tril[:], tril_dram[:])
        ident = cpool.tile([P, P], DTB)
        nc.gpsimd.dma_start(ident[:], ident_dram[:])
        bqk_t = cpool.tile([P, 8], DTF)
        nc.gpsimd.dma_start(bqk_t[:], bqk_col[:])
        if causal:
            pad_t = cpool.tile([P, NT], DTF)
            nc.gpsimd.dma_start(pad_t[:], pad01[:])
            pad_cb = cpool.tile([P, NT, HG], DTB)
            nc.gpsimd.dma_start(pad_cb[:], pad_colb[:])
        else:
            m_t = pp.tile([P, NT, S], DTB)
            for tj in range(NT):
                nc.gpsimd.dma_start(m_t[:, tj, :], maskT[:, tj, :])
        # delay the v-weight and wo DMA issue until the critical projection
        # inputs have landed: a tiny gpsimd copy depending on the last
        # xpe half-0 (resp. half-1) tile holds back the queue so the DMA
        # engines don't steal bandwidth from the first QK wave's stream
        dly = cpool.tile([1, 2], DTB)
        nc.gpsimd.tensor_copy(dly[0:1, 0:1], xpe[0:1, KT - 1, 511:512])
        for kt in range(KT):
            nc.gpsimd.dma_start(wq_b[:, kt, 2 * F:3 * F],
                                wqkvT[P * kt:P * (kt + 1), 2 * F:3 * F])
        nc.gpsimd.tensor_copy(dly[0:1, 1:2], xpe[0:1, KT - 1, 1023:1024])
        for kc in range(4):
            nc.gpsimd.dma_start(wo_b[:, kc, :], woT[P * kc:P * (kc + 1), :])

        warm_sta = cpool.tile([P, P], DTB)
        nc.vector.memset(warm_sta[:], 0.0)
        warm_rhs = cpool.tile([P, 256], DTB)
        nc.vector.memset(warm_rhs[:], 0.0)

        # doubled tril (for masking both heads of a pair in one op)
        tril2 = cpool.tile([P, 2, P], DTB)
        for x in range(2):
            nc.vector.tensor_copy(tril2[:, x, :], tril[:])
        if not causal:
            m2_t = pp.tile([P, 2, NT, S], DTB)
            for x in range(2):
                nc.vector.tensor_copy(m2_t[:, x, :, :], m_t[:])

        # ---- PE warmup: dummy matmuls during the input-DMA window release
        # the HAM clock gate and keep the p-state ramp warm before the
        # projections start ----
        with tc.tile_pool(name="warmps", bufs=1, space="PSUM") as warmps:
            wps = warmps.tile([P, 256], DTF)
            for _ in range(WARM_PRE):
                nc.tensor.matmul(wps[:], warm_sta[:], warm_rhs[:])

        v3 = v_t[:].rearrange("p n (h c) -> p n h c", c=CC + 1)
        if causal:
            nc.vector.tensor_copy(v3[:, :, :, CC], pad_cb[:])
        else:
            nc.vector.memset(v3[:, :, :, CC], 1.0)

        ep = ctx.enter_context(tc.tile_pool(name="ep", bufs=3))
        ptp = ctx.enter_context(tc.tile_pool(name="ptp",
                                             bufs=1 if causal else 4))
        osbp = ctx.enter_context(tc.tile_pool(name="osb", bufs=4))

        phase_pj = ExitStack()
        projps = phase_pj.enter_context(tc.tile_pool(name="projps", bufs=1,
                                                     space="PSUM"))

        def qk_half_wave(which, nh, pool, tagpfx):
            # project q (which=0) or k (which=1), 4 feature tiles, token
            # half nh; kt-outer so matmuls stream behind the weight DMAs
            tiles = [pool.tile([P, S // 2], DTF, tag=f"{tagpfx}{i}",
                               name=f"ps{tagpfx}{i}") for i in range(4)]
            for kt in range(KT):
                step = []
                for ft in range(4):
                    step.append(nc.tensor.matmul(
                        tiles[ft][:],
                        wq_b[:, kt, F * which + P * ft:F * which + P * (ft + 1)],
                        xpe[:, kt, (S // 2) * nh:(S // 2) * (nh + 1)],
                        start=(kt == 0), stop=(kt == KT - 1),
                    ))
                yield step
            dst = q_t if which == 0 else k_t
            for ft in range(4):
                nc.vector.tensor_scalar_add(
                    dst[:, ft, (S // 2) * nh:(S // 2) * (nh + 1)],
                    tiles[ft][:], bqk_t[:, 4 * which + ft:4 * which + ft + 1])
                yield

        def v_wave():
            # tt-outer: each token tile finishes its contraction and hands
            # off to the vector epilogue while the PE moves to the next tile
            for tt in range(8):
                vt = projps.tile([P, F], DTF, tag=f"pj{tt % 4}",
                                 name=f"psv{tt}")
                for kt in range(KT):
                    nc.tensor.matmul(
                        vt[:], xpe[:, kt, P * tt:P * (tt + 1)],
                        wq_b[:, kt, 2 * F:3 * F],
                        start=(kt == 0), stop=(kt == KT - 1),
                    )
                if causal:
                    nc.vector.tensor_scalar_mul(
                        v3[:, tt, :, 0:CC],
                        vt[:].rearrange("p (h c) -> p h c", c=CC),
                        pad_t[:, tt:tt + 1],
                    )
                else:
                    nc.vector.tensor_copy(
                        v3[:, tt, :, 0:CC],
                        vt[:].rearrange("p (h c) -> p h c", c=CC),
                    )
                yield

        pts = {}  # (qp, hp) -> exp'd score tile, [P, 2, n_t, 2P]

        def score_exp(qp, ctj, scpool):
            # scores + exp for query pair qp; the exp'd probabilities land
            # in SBUF tiles that survive until pv_out(qp) consumes them
            q0, q1 = 2 * qp, 2 * qp + 1
            n_t = q1 + 1 if causal else NT
            for hp in range(HG // 2):
                ft = hp
                pt = ptp.tile([P, 2, n_t, 2 * P], DTB,
                              tag=f"pt{qp}_{hp}" if causal else "pt",
                              name=f"pt{qp}_{hp}")
                pts[(qp, hp)] = pt
                for c0 in range(0, n_t, ctj):
                    cn = min(ctj, n_t - c0)
                    scp = scpool.tile([P, 2, ctj, 2 * P], DTF, tag="scp",
                                      name="scp")
                    for tj in range(c0, c0 + cn):
                        nc.tensor.matmul(
                            scp[:, 0, tj - c0, :],
                            k_t[0:CC, ft, P * tj:P * (tj + 1)],
                            q_t[0:CC, ft, 2 * P * qp:2 * P * (qp + 1)],
                        )
                        nc.tensor.matmul(
                            scp[:, 1, tj - c0, :],
                            k_t[CC:P, ft, P * tj:P * (tj + 1)],
                            q_t[CC:P, ft, 2 * P * qp:2 * P * (qp + 1)],
                        )
                    nc.scalar.activation(
                        pt[:, :, c0:c0 + cn, :], scp[:, :, 0:cn, :],
                        mybir.ActivationFunctionType.Exp, scale=TEMP)
                    yield

        def pv_out(qp, keep_warm=False):
            # mask, PV, normalization, transpose into attn_l for pair qp
            q0, q1 = 2 * qp, 2 * qp + 1
            for iq, qi in enumerate((q0, q1)):
                nt_i = qi + 1 if causal else NT
                # head stride padded to 128 f32 so no PV output window
                # crosses a PSUM bank boundary (2 banks, 4 heads per bank)
                out_ab = ops_pool.tile([P, HG, P], DTF,
                                       tag="outab", name="out_ab")
                for hp in range(HG // 2):
                    pt = pts[(qp, hp)]
                    if causal:
                        nc.vector.tensor_mul(
                            pt[:, :, qi, P * iq:P * (iq + 1)],
                            pt[:, :, qi, P * iq:P * (iq + 1)], tril2[:])
                    else:
                        for tj in range(nt_i):
                            nc.vector.tensor_mul(
                                pt[:, :, tj, P * iq:P * (iq + 1)],
                                pt[:, :, tj, P * iq:P * (iq + 1)],
                                m2_t[:, :, tj, P * qi:P * (qi + 1)])
                    anchor = None
                    for x, h in ((0, 2 * hp), (1, 2 * hp + 1)):
                        for tj in range(nt_i):
                            anchor = nc.tensor.matmul(
                                out_ab[:, h, 0:CC + 1],
                                pt[:, x, tj, P * iq:P * (iq + 1)],
                                v_t[:, tj, (CC + 1) * h:(CC + 1) * (h + 1)],
                                start=(tj == 0), stop=(tj == nt_i - 1),
                            )
                    yield anchor
                # normalization epilogue for this query tile (single
                # reciprocal + single scaled copy over all 8 heads)
                rec = ep.tile([P, HG], DTF, tag="rec", name="rec")
                nc.vector.reciprocal(rec[:], out_ab[:, :, CC])
                attn_s = ep.tile([P, F], DTB, tag="attn_s", name="attn_s")
                recb = rec[:, :, None].broadcast_to([P, HG, CC])
                nc.vector.tensor_tensor(
                    attn_s[:].rearrange("p (h c) -> p h c", c=CC),
                    out_ab[:, :, 0:CC],
                    recb,
                    mybir.AluOpType.mult,
                )
                tp = tpps.tile([P, HG // 2, P], DTB, tag="tp", name="tp")
                anchor = None
                for hp in range(HG // 2):
                    anchor = nc.tensor.transpose(tp[:, hp, :],
                                                 attn_s[:, P * hp:P * (hp + 1)],
                                                 ident[:])
                for hp in range(HG // 2):
                    nc.vector.tensor_copy(attn_l[:, hp, P * qi:P * (qi + 1)],
                                          tp[:, hp, :])
                yield anchor

        def out_proj(mt):
            # partial projection: local 512 channels (4 contraction tiles)
            # -> all C output columns, emitted in two 512-col halves
            psf = fo.tile([P, C // 2], DTF, tag="fo", name="psf")
            for h in range(2):
                step = []
                for kc in range(4):
                    step.append(nc.tensor.matmul(
                        psf[:], attn_l[:, kc, P * mt:P * (mt + 1)],
                        wo_b[:, kc, (C // 2) * h:(C // 2) * (h + 1)],
                        start=(kc == 0), stop=(kc == 3),
                    ))
                yield step
                osb = osbp.tile([P, C // 2], DTB, tag="osb", name="osb")
                # ACT is idle once the exps drain (~late pv2), so the late
                # tiles' PSUM->bf16 casts go there (they would otherwise
                # queue behind the pv3 epilogue on vector); the final tile
                # is emitted in quarter-width chunks to shorten the tail
                nq = 2 if mt == 7 else 1
                qw = (C // 2) // nq
                for qx in range(nq):
                    if mt >= 4:
                        nc.scalar.copy(osb[:, qw * qx:qw * (qx + 1)],
                                       psf[:, qw * qx:qw * (qx + 1)])
                    else:
                        nc.vector.tensor_copy(osb[:, qw * qx:qw * (qx + 1)],
                                              psf[:, qw * qx:qw * (qx + 1)])
                    nc.sync.dma_start(
                        out_h[P * mt:P * (mt + 1),
                              (C // 2) * h + qw * qx:(C // 2) * h + qw * (qx + 1)],
                        osb[:, qw * qx:qw * (qx + 1)])

        def weave(main_gen, fillers):
            """Run main_gen; after each of its yields, advance the current
            filler generator by one step."""
            for _ in main_gen:
                while fillers:
                    try:
                        next(fillers[0])
                        break
                    except StopIteration:
                        fillers.pop(0)
            for fg in fillers:
                for _ in fg:
                    pass
            fillers.clear()

        def run(gen):
            for _ in gen:
                pass

        def chain(*gens):
            for g in gens:
                yield from g

        def interleave(ga, gb):
            ita, itb = iter(ga), iter(gb)
            while True:
                done = 0
                for it in (ita, itb):
                    try:
                        next(it)
                    except StopIteration:
                        done += 1
                if done == 2:
                    return
                yield

        if causal:
            # fused first wave: q and k for token half 0 (8 PSUM banks)
            with tc.tile_pool(name="pkps", bufs=1, space="PSUM") as pkps:
                run(interleave(qk_half_wave(0, 0, projps, "pj"),
                               qk_half_wave(1, 0, pkps, "pk")))
            # scores/exp of pairs 0-2 overlap the half-1 q/k and V waves
            with tc.tile_pool(name="scps", bufs=2, space="PSUM") as scps:
                weave(chain(score_exp(0, 2, scps), score_exp(1, 2, scps)),
                      [qk_half_wave(0, 1, projps, "pj"),
                       qk_half_wave(1, 1, projps, "pj")])
                weave(score_exp(2, 2, scps), [v_wave()])
            phase_pj.close()
            # PV / normalization / output projections; pair-3 scores/exp and
            # the out_projs fill PE bubbles
            ops_pool = ctx.enter_context(tc.tile_pool(name="ops", bufs=1,
                                                      space="PSUM"))
            tpps = ctx.enter_context(tc.tile_pool(name="tpps", bufs=1,
                                                  space="PSUM"))
            fo = ctx.enter_context(tc.tile_pool(name="fo", bufs=1,
                                                space="PSUM"))
            scps2 = ctx.enter_context(tc.tile_pool(name="scps2", bufs=2,
                                                   space="PSUM"))
            se3 = score_exp(3, 2, scps2)
            next(se3)
            next(se3)
            weave(chain(pv_out(0), pv_out(1), pv_out(2)),
                  [se3,
                   out_proj(0), out_proj(1), out_proj(2), out_proj(3)])
            weave(pv_out(3), [out_proj(4), out_proj(5), out_proj(6)])
            run(out_proj(7))
        else:
            # simple sequential structure (correctness fallback)
            with tc.tile_pool(name="pkps", bufs=1, space="PSUM") as pkps:
                run(interleave(qk_half_wave(0, 0, projps, "pj"),
                               qk_half_wave(1, 0, pkps, "pk")))
            run(qk_half_wave(0, 1, projps, "pj"))
            run(qk_half_wave(1, 1, projps, "pj"))
            run(v_wave())
            phase_pj.close()
            scps = ctx.enter_context(tc.tile_pool(name="scps", bufs=2,
                                                  space="PSUM"))
            ops_pool = ctx.enter_context(tc.tile_pool(name="ops", bufs=1,
                                                      space="PSUM"))
            tpps = ctx.enter_context(tc.tile_pool(name="tpps", bufs=1,
                                                  space="PSUM"))
            fo = ctx.enter_context(tc.tile_pool(name="fo", bufs=1,
                                                space="PSUM"))
            for qp in range(4):
                run(score_exp(qp, 2, scps))
                run(pv_out(qp))
            for mt in range(8):
                run(out_proj(mt))

    nc.compile()
    return nc


def _get_nc(causal: bool):
    if causal not in _NC_CACHE:
        _NC_CACHE[causal] = _build(causal)
    return _NC_CACHE[causal]


def kernel(x, pe, content_mask, padding_mask, Wqkv, bqkv, Wo, bo):
    global LAST_RESULT
    x = np.asarray(x, dtype=np.float32)
    pe = np.asarray(pe, dtype=np.float32)
    content_mask = np.asarray(content_mask, dtype=bool)
    padding_mask = np.asarray(padding_mask, dtype=bool)
    Wqkv = np.asarray(Wqkv, dtype=np.float32)
    bqkv = np.asarray(bqkv, dtype=np.float32)
    Wo = np.asarray(Wo, dtype=np.float32)
    bo = np.asarray(bo, dtype=np.float32)
    assert x.shape == (S, B, C) and Wqkv.shape == (3 * C, C)

    causal_2d = np.triu(np.ones((S, S), dtype=bool), 1)
    causal = np.array_equal(content_mask,
                            np.broadcast_to(causal_2d[:, :, None], (S, S, B)))

    nc = _get_nc(causal)

    xpe_full = x + pe
    bv_full = bqkv[2 * C:3 * C]
    obias = bo + Wo @ bv_full  # [C], f32

    in_maps = []
    for core in range(N_CORES):
        b, hg = core // 2, core % 2
        m = {
            "xpeT": np.ascontiguousarray(xpe_full[:, b, :].T.astype(BF16)),
        }
        rows = np.concatenate([
            np.arange(F * hg, F * (hg + 1)),
            np.arange(C + F * hg, C + F * (hg + 1)),
            np.arange(2 * C + F * hg, 2 * C + F * (hg + 1)),
        ])
        m["wqkvT"] = np.ascontiguousarray(Wqkv[rows, :].T.astype(BF16))
        m["woT"] = np.ascontiguousarray(
            Wo[:, F * hg:F * (hg + 1)].T.astype(BF16))
        bq = bqkv[F * hg:F * (hg + 1)]
        bk = bqkv[C + F * hg:C + F * (hg + 1)]
        bqk = np.concatenate([bq, bk]).reshape(8, P).T  # [P, 8]
        m["bqk_col"] = np.ascontiguousarray(bqk.astype(np.float32))
        if causal:
            keep = (~padding_mask[:, b]).astype(np.float32)  # [S]
            m["pad01"] = np.ascontiguousarray(keep.reshape(NT, P).T)
            m["pad_colb"] = np.ascontiguousarray(np.broadcast_to(
                m["pad01"][:, :, None], (P, NT, HG)).astype(BF16))
        else:
            keep2d = ~(content_mask[:, :, b] | padding_mask[None, :, b])  # [s, t]
            mT = keep2d.T.astype(BF16)  # [t, s]
            m["maskT"] = np.ascontiguousarray(mT.reshape(NT, P, S).transpose(1, 0, 2))
        in_maps.append(m)

    trace = bool(os.environ.get("BASS_KERNEL_TRACE"))
    res = run_bass_kernel_spmd(nc, in_maps, core_ids=list(range(N_CORES)),
                               trace=trace)
    LAST_RESULT = res

    out = np.empty((S, B, C), dtype=np.float32)
    for b in range(B):
        p0 = res.results[2 * b]["out"].astype(np.float32)
        p1 = res.results[2 * b + 1]["out"].astype(np.float32)
        out[:, b, :] = p0 + p1 + obias[None, :]
    return out
